# revision 20
# baseline (speedup 1.0000x reference)
"""Trainium2 Bass kernel for nn_Attention_20298015441502 (sparse local attention).

Model: RMSNorm -> fused QKV -> (bug-faithful head-indexed RoPE) -> banded local
attention (window 256) -> out-proj -> residual.

Sharding (8 cores): data-parallel over batch (2) x tensor-parallel over heads
(4 groups of 4 heads). Core c = b*4+g computes batch b, heads [4g, 4g+4).
Out-proj is row-split; the host sums the 4 partials per batch and adds the
residual (that sum is the TP unshard step).

Host-side algebraic folds (exact reformulations):
 - The reference's RoPE angle depends on head index, not position, so the
   rotation is a constant orthogonal transform per head: fold into q/k weights.
 - norm_scale and the 1/sqrt(D) score scale fold into the weights too.
Device computes: rstd from x (ACT square + ones-matmul over partitions);
qkvT = W'^T x^T scaled by a PE-broadcast rstd row; per 128-key tile, transposed
scores k^T q over its <=5 attending 128-query tiles; p = exp(scores) in bf16
(scores are O(4): no max subtraction) with triangular masks on the two edge
tiles; o^T plus softmax sums via an appended ones-row on v (bf16 matmuls);
normalize by the broadcast reciprocal sums; out^T = woutT^T o^T.
"""

import sys

sys.path.insert(0, "/opt/trn_rl_repo")

import numpy as np
import ml_dtypes

import concourse.bass as bass
import concourse.bacc as bacc
import concourse.mybir as mybir
from concourse.bass import _add_dep_helper
from concourse.bass_utils import run_bass_kernel_spmd
from concourse.bass_interp import get_hw_module
from concourse.tile import TileContext

F32 = mybir.dt.float32
F32R = mybir.dt.float32r
BF16 = mybir.dt.bfloat16

SIZE = 1024
HEADS = 16
HEAD = 64
EPS = 1e-5
ROPE_BASE = 10000.0
B, S = 2, 4096
HL = 4            # heads per core
NT = S // 128     # 32 s-tiles
NSG = S // 512    # 8 s-groups
CT = SIZE // 128  # 8 c-tiles
NL = 3 * HL * HEAD // 128   # 6 local qkv n-tiles
VB = 80                     # padded per-head v block (64 v + 1 ones + pad), 32B-aligned
VW = HL * VB                # v block width per s-tile

_cached = {}


def _build_module():
    nc = bacc.Bacc("TRN2", target_bir_lowering=False, debug=False)
    AF = mybir.ActivationFunctionType

    xT_d = nc.dram_tensor("xT", [SIZE, S], BF16, kind="ExternalInput").ap()
    wqkvT_d = nc.dram_tensor("wqkvT", [SIZE, 3 * HL * HEAD], BF16, kind="ExternalInput").ap()
    woutT_d = nc.dram_tensor("woutT", [HL * HEAD, SIZE], BF16, kind="ExternalInput").ap()
    onescol_d = nc.dram_tensor("onescol", [128, 1], BF16, kind="ExternalInput").ap()
    onesrow_d = nc.dram_tensor("onesrow", [1, 128], F32R, kind="ExternalInput").ap()
    mlo_d = nc.dram_tensor("mlo", [128, 128], BF16, kind="ExternalInput").ap()
    mhi_d = nc.dram_tensor("mhi", [128, 128], BF16, kind="ExternalInput").ap()
    outT_d = nc.dram_tensor("outT", [SIZE, S], F32, kind="ExternalOutput").ap()
    dbg = {}
    if True:
        for nm, shp, dt_ in [("d_qk0", [128, S], BF16), ("d_vsb", [128, NT * VW], BF16),
                             ("d_osb0", [128, S], F32), ("d_onrm0", [128, S], BF16),
                             ("d_inv", [HL, S], F32)]:
            dbg[nm] = nc.dram_tensor(nm, shp, dt_, kind="ExternalOutput").ap()
    inv_b = nc.dram_tensor("inv_bounce", [HL, S], F32)
    sums_b = nc.dram_tensor("sums_bounce", [HL, S], F32)
    rstd_b = nc.dram_tensor("rstd_bounce", [1, S], F32R)
    sqs_b = nc.dram_tensor("sqs_bounce", [1, S], F32)

    with TileContext(nc) as tc:
        with tc.tile_pool(name="consts", bufs=1) as consts:
            onescol = consts.tile([128, 1], BF16)
            nc.sync.dma_start(onescol, onescol_d)
            onesrow = consts.tile([1, 128], F32R)
            nc.sync.dma_start(onesrow, onesrow_d)
            mlo = consts.tile([128, 128], BF16)
            nc.sync.dma_start(mlo, mlo_d)
            mhi = consts.tile([128, 128], BF16)
            nc.sync.dma_start(mhi, mhi_d)
            epst = consts.tile([1, 1], F32)
            nc.vector.memset(epst, EPS)

            # persistent activations: q/k and v||ones in bf16
            qk_sb = [consts.tile([128, S], BF16, tag=f"qk{n}", name=f"qk{n}")
                     for n in range(4)]
            v_sb = consts.tile([128, NT * VW], BF16)
            ones_ap = bass.AP(tensor=v_sb.tensor, offset=v_sb.offset + HEAD,
                              ap=[v_sb.ap[0], [VB, NT * HL], [1, 1]])
            nc.vector.memset(ones_ap, 1.0)

            # ---------------- Phase 1: rstd + QKV (+ v transpose) ----------------
            vtp_cm = tc.tile_pool(name="vt", bufs=2)
            vtp = vtp_cm.__enter__()
            with tc.tile_pool(name="wq", bufs=1) as wqp, \
                 tc.tile_pool(name="xin", bufs=1) as xin, \
                 tc.tile_pool(name="sq", bufs=3) as sqp, \
                 tc.tile_pool(name="bcs", bufs=1) as bcsp:
                wq_sb = [wqp.tile([128, NL * 128], BF16, tag=f"w{c}", name=f"w{c}")
                         for c in range(CT)]
                for c in range(CT):
                    eng = nc.sync if c % 2 == 0 else nc.gpsimd
                    eng.dma_start(wq_sb[c], wqkvT_d[c * 128:(c + 1) * 128, :])
                xts = [xin.tile([128, S], BF16, tag=f"x{c}", name=f"x{c}")
                       for c in range(CT)]
                for c in range(CT):
                    eng = nc.sync if c % 2 == 0 else nc.gpsimd
                    eng.dma_start(xts[c], xT_d[c * 128:(c + 1) * 128, :])

                # pass A: rstd rows; reciprocal via (128,32) DRAM reshape
                ps_a_cm = tc.tile_pool(name="ps_a", bufs=2, space="PSUM")
                ps_a = ps_a_cm.__enter__()
                sq_writes = []
                for sg in range(NSG):
                    ssl = slice(sg * 512, (sg + 1) * 512)
                    ssps = ps_a.tile([1, 512], F32, tag="ssps")
                    for c in range(CT):
                        sq = sqp.tile([128, 512], BF16)
                        nc.vector.tensor_mul(sq, xts[c][:, ssl], xts[c][:, ssl])
                        nc.tensor.matmul(ssps, onescol, sq,
                                         start=(c == 0), stop=(c == CT - 1))
                    sqs = bcsp.tile([1, 512], F32, tag="sqs", bufs=3)
                    nc.scalar.activation(sqs, ssps, AF.Sqrt, bias=epst,
                                         scale=1.0 / SIZE)
                    sqw = nc.sync.dma_start(sqs_b[0, ssl], sqs)
                    sq_writes.append(sqw)
                rsp = bcsp.tile([128, S // 128], F32, tag="rsp")
                rspr = nc.sync.dma_start(
                    rsp, bass.AP(tensor=sqs_b, offset=0,
                                 ap=[[S // 128, 128], [1, S // 128]]))
                for w in sq_writes:
                    _add_dep_helper(rspr.ins, w.ins, sync=True,
                                    reason="sqs gather after all sq writes")
                rspi = bcsp.tile([128, S // 128], F32R, tag="rspi")
                with nc.allow_low_precision("f32r rounding for PE broadcast"):
                    nc.vector.reciprocal(rspi, rsp)
                rbw = nc.sync.dma_start(
                    bass.AP(tensor=rstd_b, offset=0,
                            ap=[[S // 128, 128], [1, S // 128]]), rspi)
                rstd_sb = bcsp.tile([1, S], F32R, tag="rstd_sb")
                rbr = nc.sync.dma_start(rstd_sb, rstd_b[0:1, :])
                _add_dep_helper(rbr.ins, rbw.ins, sync=True,
                                reason="rstd read after write")
                bcsbs = []
                for sg in range(NSG):
                    ssl = slice(sg * 512, (sg + 1) * 512)
                    bcps = ps_a.tile([128, 512], F32, tag="bcps")
                    nc.tensor.matmul(bcps, onesrow, rstd_sb[:, ssl],
                                     start=True, stop=True)
                    bcsb = bcsp.tile([128, 512], F32, tag=f"bcsb{sg}",
                                     name=f"bcsb{sg}")
                    nc.scalar.copy(bcsb, bcps)
                    bcsbs.append(bcsb)

                ps_a_cm.__exit__(None, None, None)
                # pass B: qkv, n-outer so each weight block is loaded once
                ps_qkv_cm = tc.tile_pool(name="ps_qkv", bufs=8, space="PSUM")
                ps_qkv = ps_qkv_cm.__enter__()
                for n in range(NL):
                    mms = [ps_qkv.tile([128, 512], F32, tag="mm", name="mm")
                           for _ in range(NSG)]
                    for c in range(CT):
                        for sg in range(NSG):
                            nc.tensor.matmul(
                                mms[sg], wq_sb[c][:, n * 128:(n + 1) * 128],
                                xts[c][:, sg * 512:(sg + 1) * 512],
                                start=(c == 0), stop=(c == CT - 1))
                    for sg in range(NSG):
                        ssl = slice(sg * 512, (sg + 1) * 512)
                        if n < 4:
                            nc.vector.tensor_mul(qk_sb[n][:, ssl], mms[sg],
                                                 bcsbs[sg])
                        else:
                            vtt = vtp.tile([128, 512], BF16, tag=f"vtt{n-4}")
                            nc.vector.tensor_mul(vtt, mms[sg], bcsbs[sg])
                            for u in range(4):
                                st = sg * 4 + u
                                for hh in range(2):
                                    h = (n - 4) * 2 + hh
                                    dst = v_sb[:, st * VW + h * VB:
                                               st * VW + h * VB + HEAD]
                                    nc.sync.dma_start_transpose(
                                        dst, vtt[hh * 64:hh * 64 + 64,
                                                 u * 128:(u + 1) * 128])
                ps_qkv_cm.__exit__(None, None, None)

            # -------- Phase 2: banded attention + normalize + out-proj --------
            with tc.tile_pool(name="atp", bufs=1) as atp, \
                 tc.tile_pool(name="wo", bufs=1) as wop:
                oT_sb = [atp.tile([128, S], F32, tag=f"osb{p}", name=f"osb{p}")
                         for p in range(2)]
                onrm = [atp.tile([128, S], BF16, tag=f"onrm{p}", name=f"onrm{p}")
                        for p in range(2)]
                wo_sb = [wop.tile([128, SIZE], BF16, tag=f"wo{k}", name=f"wo{k}")
                         for k in range(2)]
                for k in range(2):
                    nc.gpsimd.dma_start(wo_sb[k], woutT_d[k * 128:(k + 1) * 128, :])

                ptiles = {}
                with tc.tile_pool(name="pt", bufs=9) as ptp, \
                     tc.tile_pool(name="stg", bufs=4) as stgp, \
                     tc.tile_pool(name="ibc", bufs=3) as ibcp, \
                     tc.tile_pool(name="ost", bufs=4) as ostp, \
                     tc.tile_pool(name="ps_sc", bufs=2, space="PSUM") as ps_sc, \
                     tc.tile_pool(name="ps_ot", bufs=1, space="PSUM") as ps_ot, \
                     tc.tile_pool(name="ps_op", bufs=2, space="PSUM") as ps_op:

                    def emit_group(qg):
                        # o-matmuls for q-tiles 4qg..4qg+3, k2-major so the
                        # stationary v block is reused across regions
                        t0, t3 = 4 * qg, 4 * qg + 3
                        klo_g, khi_g = max(t0 - 2, 0), min(t3 + 2, NT - 1)
                        ots = [ps_ot.tile([HEAD + 1, 512], F32, tag=f"ot{h}",
                                          name=f"ot{h}") for h in range(HL)]
                        # k2 = t0+1 attends the whole group (N=512): do it first
                        # with start=True since start clears the full psum bank.
                        k2s = [t0 + 1] + [k for k in range(klo_g, khi_g + 1)
                                          if k != t0 + 1]
                        for ki, k2 in enumerate(k2s):
                            ts = [t for t in range(t0, t3 + 1)
                                  if k2 - 2 <= t <= k2 + 2]
                            for h in range(HL):
                                pt, qlo2 = ptiles[(k2, h)]
                                vs = v_sb[:, k2 * VW + h * VB:
                                          k2 * VW + h * VB + HEAD + 1]
                                ja, jb = ts[0] - t0, ts[-1] - t0
                                off = (ts[0] - qlo2) * 128
                                nc.tensor.matmul(
                                    ots[h][:, ja * 128:(jb + 1) * 128],
                                    vs, pt[:, off:off + 128 * len(ts)],
                                    start=(ki == 0), stop=(ki == len(k2s) - 1),
                                    skip_group_check=True)
                        gsl = slice(qg * 512, (qg + 1) * 512)
                        stage_ws = []
                        for h in range(HL):
                            pair = h // 2
                            stage = stgp.tile([1, 512], F32, tag="stage")
                            nc.scalar.copy(stage, ots[h][HEAD:HEAD + 1, :])
                            sw = nc.sync.dma_start(sums_b[h, gsl], stage)
                            stage_ws.append(sw)
                            nc.scalar.copy(
                                oT_sb[pair][(h % 2) * 64:(h % 2) * 64 + 64, gsl],
                                ots[h][0:HEAD, :])
                        # reciprocal of the 4x512 sums as a (128,16) tile
                        sgth = stgp.tile([128, 16], F32, tag="sgth")
                        sgr = nc.sync.dma_start(
                            sgth, bass.AP(tensor=sums_b, offset=qg * 512,
                                          ap=[[S, 4], [16, 32], [1, 16]]))
                        for w in stage_ws:
                            _add_dep_helper(sgr.ins, w.ins, sync=True,
                                            reason="sums gather after stage writes")
                        sgti = stgp.tile([128, 16], F32, tag="sgti")
                        nc.vector.reciprocal(sgti, sgth)
                        invw = nc.sync.dma_start(
                            bass.AP(tensor=inv_b, offset=qg * 512,
                                    ap=[[S, 4], [16, 32], [1, 16]]), sgti)
                        for pair in range(2):
                            ibsb = ibcp.tile([128, 512], F32, tag="ibsb")
                            src_ap = bass.AP(tensor=inv_b,
                                             offset=pair * 2 * S + qg * 512,
                                             ap=[[S, 2], [0, 64], [1, 512]])
                            nc.sync.dma_start(ibsb, src_ap)
                            nc.vector.tensor_mul(onrm[pair][:, gsl],
                                                 oT_sb[pair][:, gsl], ibsb)
                        for ct in range(CT):
                            ops = ps_op.tile([128, 512], F32)
                            nc.tensor.matmul(ops, wo_sb[0][:, ct * 128:(ct + 1) * 128],
                                             onrm[0][:, gsl], start=True, stop=False)
                            nc.tensor.matmul(ops, wo_sb[1][:, ct * 128:(ct + 1) * 128],
                                             onrm[1][:, gsl], start=False, stop=True)
                            ot = ostp.tile([128, 512], F32)
                            if ct % 2 == 0:
                                nc.scalar.copy(ot, ops)
                            else:
                                nc.vector.tensor_copy(ot, ops)
                            nc.sync.dma_start(outT_d[ct * 128:(ct + 1) * 128, gsl], ot)

                    for kt in range(NT):
                        qlo, qhi = max(kt - 2, 0), min(kt + 2, NT - 1)
                        span = (qhi - qlo + 1) * 128
                        chunks = ([(0, 384), (384, span - 384)] if span > 512
                                  else [(0, span)])
                        for h in range(HL):
                            ktile = qk_sb[2 + h // 2][(h % 2) * 64:(h % 2) * 64 + 64,
                                                      kt * 128:(kt + 1) * 128]
                            pt = ptp.tile([128, 640], BF16, tag=f"pt{h}")
                            for (c0, w) in chunks:
                                sps = ps_sc.tile([128, 512], F32)
                                qtile = qk_sb[h // 2][
                                    (h % 2) * 64:(h % 2) * 64 + 64,
                                    qlo * 128 + c0: qlo * 128 + c0 + w]
                                nc.tensor.matmul(sps[:, :w], ktile, qtile,
                                                 start=True, stop=True)
                                nc.scalar.activation(pt[:, c0:c0 + w], sps[:, :w],
                                                     AF.Exp)
                            if kt >= 2:
                                nc.vector.tensor_mul(pt[:, 0:128], pt[:, 0:128], mlo)
                            if kt <= NT - 3:
                                nc.vector.tensor_mul(pt[:, span - 128:span],
                                                     pt[:, span - 128:span], mhi)
                            ptiles[(kt, h)] = (pt, qlo)
                        for qg in range(NT // 4):
                            if kt == min(4 * qg + 5, NT - 1):
                                emit_group(qg)
                    if dbg:
                        nc.sync.dma_start(dbg["d_qk0"], qk_sb[0])
                        nc.sync.dma_start(dbg["d_vsb"], v_sb)
                        nc.sync.dma_start(dbg["d_osb0"], oT_sb[0])
                        nc.sync.dma_start(dbg["d_onrm0"], onrm[0])
                        nc.sync.dma_start(dbg["d_inv"], inv_b[:, :])
            vtp_cm.__exit__(None, None, None)

    nc.compile()
    nc.m = get_hw_module(nc.m)
    return nc


def _rope_cos_sin(n, d):
    inv_freq = 1.0 / (ROPE_BASE ** (np.arange(0, d, 2, dtype=np.float32) / d))
    freqs = np.arange(n, dtype=np.float32)[:, None] * inv_freq[None, :]
    emb = np.concatenate([freqs, freqs], axis=-1)
    return np.cos(emb).astype(np.float32), np.sin(emb).astype(np.float32)


def _prep_inputs(x, w_qkv, w_out, norm_scale):
    cos, sin = _rope_cos_sin(HEADS, HEAD)
    Wf = np.ascontiguousarray(w_qkv.reshape(3, HEADS, HEAD, SIZE)).astype(np.float32)
    d2 = HEAD // 2

    def rot(Wh, h):
        out = np.empty_like(Wh)
        out[:d2] = cos[h, :d2, None] * Wh[:d2] - sin[h, :d2, None] * Wh[d2:]
        out[d2:] = cos[h, d2:, None] * Wh[d2:] + sin[h, d2:, None] * Wh[:d2]
        return out

    scale = np.float32(1.0 / np.sqrt(HEAD))
    Wq = np.stack([rot(Wf[0, h], h) for h in range(HEADS)]) * scale
    Wk = np.stack([rot(Wf[1, h], h) for h in range(HEADS)])
    Wv = Wf[2]
    ns = norm_scale.astype(np.float32)[None, None, :]
    Wq = Wq * ns
    Wk = Wk * ns
    Wv = Wv * ns

    i = np.arange(128)
    mlo = (i[:, None] <= i[None, :]).astype(ml_dtypes.bfloat16)
    mhi = (i[:, None] >= i[None, :]).astype(ml_dtypes.bfloat16)

    in_maps = []
    for core in range(8):
        b, g = divmod(core, 4)
        hs = slice(HL * g, HL * g + HL)
        wq_l = Wq[hs].reshape(HL * HEAD, SIZE)
        wk_l = Wk[hs].reshape(HL * HEAD, SIZE)
        wv_l = Wv[hs].reshape(HL * HEAD, SIZE)
        wqkvT = np.ascontiguousarray(np.concatenate([wq_l, wk_l, wv_l], 0).T)
        woutT = np.ascontiguousarray(
            w_out[:, HL * HEAD * g: HL * HEAD * (g + 1)].T.astype(np.float32))
        in_maps.append({
            "xT": np.ascontiguousarray(x[b].T).astype(ml_dtypes.bfloat16),
            "wqkvT": wqkvT.astype(ml_dtypes.bfloat16),
            "woutT": woutT.astype(ml_dtypes.bfloat16),
            "onescol": np.ones((128, 1), ml_dtypes.bfloat16),
            "onesrow": np.ones((1, 128), np.float32),
            "mlo": mlo,
            "mhi": mhi,
        })
    return in_maps


def _run(in_maps, trace=False, **kw):
    if "nc" not in _cached:
        _cached["nc"] = _build_module()
    return run_bass_kernel_spmd(_cached["nc"], in_maps, core_ids=list(range(8)),
                                trace=trace, **kw)


def kernel(x, mask, w_qkv, w_out, norm_scale):
    x = np.asarray(x)
    in_maps = _prep_inputs(x, np.asarray(w_qkv), np.asarray(w_out),
                           np.asarray(norm_scale))
    res = _run(in_maps)
    out = np.empty((B, S, SIZE), np.float32)
    for b in range(B):
        acc = res.results[b * 4]["outT"].copy()
        for g in range(1, 4):
            acc += res.results[b * 4 + g]["outT"]
        out[b] = acc.T + x[b]
    return out


# revision 21
# speedup vs baseline: 1.0007x; 1.0007x over previous
"""Trainium2 Bass kernel for nn_Attention_20298015441502 (sparse local attention).

Model: RMSNorm -> fused QKV -> (bug-faithful head-indexed RoPE) -> banded local
attention (window 256) -> out-proj -> residual.

Sharding (8 cores): data-parallel over batch (2) x tensor-parallel over heads
(4 groups of 4 heads). Core c = b*4+g computes batch b, heads [4g, 4g+4).
Out-proj is row-split; the host sums the 4 partials per batch and adds the
residual (that sum is the TP unshard step).

Host-side algebraic folds (exact reformulations):
 - The reference's RoPE angle depends on head index, not position, so the
   rotation is a constant orthogonal transform per head: fold into q/k weights.
 - norm_scale and the 1/sqrt(D) score scale fold into the weights too.
Device computes: rstd from x (ACT square + ones-matmul over partitions);
qkvT = W'^T x^T scaled by a PE-broadcast rstd row; per 128-key tile, transposed
scores k^T q over its <=5 attending 128-query tiles; p = exp(scores) in bf16
(scores are O(4): no max subtraction) with triangular masks on the two edge
tiles; o^T plus softmax sums via an appended ones-row on v (bf16 matmuls);
normalize by the broadcast reciprocal sums; out^T = woutT^T o^T.
"""

import sys

sys.path.insert(0, "/opt/trn_rl_repo")

import numpy as np
import ml_dtypes

import concourse.bass as bass
import concourse.bacc as bacc
import concourse.mybir as mybir
from concourse.bass import _add_dep_helper
from concourse.bass_utils import run_bass_kernel_spmd
from concourse.bass_interp import get_hw_module
from concourse.tile import TileContext

F32 = mybir.dt.float32
F32R = mybir.dt.float32r
BF16 = mybir.dt.bfloat16

SIZE = 1024
HEADS = 16
HEAD = 64
EPS = 1e-5
ROPE_BASE = 10000.0
B, S = 2, 4096
HL = 4            # heads per core
NT = S // 128     # 32 s-tiles
NSG = S // 512    # 8 s-groups
CT = SIZE // 128  # 8 c-tiles
NL = 3 * HL * HEAD // 128   # 6 local qkv n-tiles
VB = 80                     # padded per-head v block (64 v + 1 ones + pad), 32B-aligned
VW = HL * VB                # v block width per s-tile

_cached = {}


def _build_module():
    nc = bacc.Bacc("TRN2", target_bir_lowering=False, debug=False)
    AF = mybir.ActivationFunctionType

    xT_d = nc.dram_tensor("xT", [SIZE, S], BF16, kind="ExternalInput").ap()
    wqkvT_d = nc.dram_tensor("wqkvT", [SIZE, 3 * HL * HEAD], BF16, kind="ExternalInput").ap()
    woutT_d = nc.dram_tensor("woutT", [HL * HEAD, SIZE], BF16, kind="ExternalInput").ap()
    onescol_d = nc.dram_tensor("onescol", [128, 1], BF16, kind="ExternalInput").ap()
    onesrow_d = nc.dram_tensor("onesrow", [1, 128], F32R, kind="ExternalInput").ap()
    mlo_d = nc.dram_tensor("mlo", [128, 128], BF16, kind="ExternalInput").ap()
    mhi_d = nc.dram_tensor("mhi", [128, 128], BF16, kind="ExternalInput").ap()
    outT_d = nc.dram_tensor("outT", [SIZE, S], F32, kind="ExternalOutput").ap()
    dbg = {}
    if True:
        for nm, shp, dt_ in [("d_qk0", [128, S], BF16), ("d_vsb", [128, NT * VW], BF16),
                             ("d_osb0", [128, S], F32), ("d_onrm0", [128, S], BF16),
                             ("d_inv", [HL, S], F32)]:
            dbg[nm] = nc.dram_tensor(nm, shp, dt_, kind="ExternalOutput").ap()
    inv_b = nc.dram_tensor("inv_bounce", [HL, S], F32)
    sums_b = nc.dram_tensor("sums_bounce", [HL, S], F32)
    rstd_b = nc.dram_tensor("rstd_bounce", [1, S], F32R)
    sqs_b = nc.dram_tensor("sqs_bounce", [1, S], F32)

    with TileContext(nc) as tc:
        with tc.tile_pool(name="consts", bufs=1) as consts:
            onescol = consts.tile([128, 1], BF16)
            nc.sync.dma_start(onescol, onescol_d)
            onesrow = consts.tile([1, 128], F32R)
            nc.sync.dma_start(onesrow, onesrow_d)
            mlo = consts.tile([128, 128], BF16)
            nc.sync.dma_start(mlo, mlo_d)
            mhi = consts.tile([128, 128], BF16)
            nc.sync.dma_start(mhi, mhi_d)
            epst = consts.tile([1, 1], F32)
            nc.vector.memset(epst, EPS)

            # persistent activations: q/k and v||ones in bf16
            qk_sb = [consts.tile([128, S], BF16, tag=f"qk{n}", name=f"qk{n}")
                     for n in range(4)]
            v_sb = consts.tile([128, NT * VW], BF16)
            ones_ap = bass.AP(tensor=v_sb.tensor, offset=v_sb.offset + HEAD,
                              ap=[v_sb.ap[0], [VB, NT * HL], [1, 1]])
            nc.vector.memset(ones_ap, 1.0)

            # ---------------- Phase 1: rstd + QKV (+ v transpose) ----------------
            vtp_cm = tc.tile_pool(name="vt", bufs=2)
            vtp = vtp_cm.__enter__()
            with tc.tile_pool(name="wq", bufs=1) as wqp, \
                 tc.tile_pool(name="xin", bufs=1) as xin, \
                 tc.tile_pool(name="sq", bufs=3) as sqp, \
                 tc.tile_pool(name="bcs", bufs=1) as bcsp:
                wq_sb = [wqp.tile([128, NL * 128], BF16, tag=f"w{c}", name=f"w{c}")
                         for c in range(CT)]
                for c in range(CT):
                    eng = nc.sync if c % 2 == 0 else nc.gpsimd
                    eng.dma_start(wq_sb[c], wqkvT_d[c * 128:(c + 1) * 128, :])
                xts = [xin.tile([128, S], BF16, tag=f"x{c}", name=f"x{c}")
                       for c in range(CT)]
                for c in range(CT):
                    eng = nc.sync if c % 2 == 0 else nc.gpsimd
                    eng.dma_start(xts[c], xT_d[c * 128:(c + 1) * 128, :])

                # pass A: rstd rows; reciprocal via (128,32) DRAM reshape
                ps_a_cm = tc.tile_pool(name="ps_a", bufs=2, space="PSUM")
                ps_a = ps_a_cm.__enter__()
                sq_writes = []
                for sg in range(NSG):
                    ssl = slice(sg * 512, (sg + 1) * 512)
                    ssps = ps_a.tile([1, 512], F32, tag="ssps")
                    for c in range(CT):
                        sq = sqp.tile([128, 512], BF16)
                        nc.vector.tensor_mul(sq, xts[c][:, ssl], xts[c][:, ssl])
                        nc.tensor.matmul(ssps, onescol, sq,
                                         start=(c == 0), stop=(c == CT - 1))
                    sqs = bcsp.tile([1, 512], F32, tag="sqs", bufs=3)
                    nc.scalar.activation(sqs, ssps, AF.Sqrt, bias=epst,
                                         scale=1.0 / SIZE)
                    sqw = nc.sync.dma_start(sqs_b[0, ssl], sqs)
                    sq_writes.append(sqw)
                rsp = bcsp.tile([128, S // 128], F32, tag="rsp")
                rspr = nc.sync.dma_start(
                    rsp, bass.AP(tensor=sqs_b, offset=0,
                                 ap=[[S // 128, 128], [1, S // 128]]))
                for w in sq_writes:
                    _add_dep_helper(rspr.ins, w.ins, sync=True,
                                    reason="sqs gather after all sq writes")
                rspi = bcsp.tile([128, S // 128], F32R, tag="rspi")
                with nc.allow_low_precision("f32r rounding for PE broadcast"):
                    nc.vector.reciprocal(rspi, rsp)
                rbw = nc.sync.dma_start(
                    bass.AP(tensor=rstd_b, offset=0,
                            ap=[[S // 128, 128], [1, S // 128]]), rspi)
                rstd_sb = bcsp.tile([1, S], F32R, tag="rstd_sb")
                rbr = nc.sync.dma_start(rstd_sb, rstd_b[0:1, :])
                _add_dep_helper(rbr.ins, rbw.ins, sync=True,
                                reason="rstd read after write")
                bcsbs = []
                for sg in range(NSG):
                    ssl = slice(sg * 512, (sg + 1) * 512)
                    bcps = ps_a.tile([128, 512], F32, tag="bcps")
                    nc.tensor.matmul(bcps, onesrow, rstd_sb[:, ssl],
                                     start=True, stop=True)
                    bcsb = bcsp.tile([128, 512], F32, tag=f"bcsb{sg}",
                                     name=f"bcsb{sg}")
                    nc.scalar.copy(bcsb, bcps)
                    bcsbs.append(bcsb)

                ps_a_cm.__exit__(None, None, None)
                # pass B: qkv, n-outer so each weight block is loaded once
                ps_qkv_cm = tc.tile_pool(name="ps_qkv", bufs=8, space="PSUM")
                ps_qkv = ps_qkv_cm.__enter__()
                for n in range(NL):
                    mms = [ps_qkv.tile([128, 512], F32, tag="mm", name="mm")
                           for _ in range(NSG)]
                    for c in range(CT):
                        for sg in range(NSG):
                            nc.tensor.matmul(
                                mms[sg], wq_sb[c][:, n * 128:(n + 1) * 128],
                                xts[c][:, sg * 512:(sg + 1) * 512],
                                start=(c == 0), stop=(c == CT - 1))
                    for sg in range(NSG):
                        ssl = slice(sg * 512, (sg + 1) * 512)
                        if n < 4:
                            nc.vector.tensor_mul(qk_sb[n][:, ssl], mms[sg],
                                                 bcsbs[sg])
                        else:
                            vtt = vtp.tile([128, 512], BF16, tag=f"vtt{n-4}")
                            nc.vector.tensor_mul(vtt, mms[sg], bcsbs[sg])
                            for u in range(4):
                                st = sg * 4 + u
                                for hh in range(2):
                                    h = (n - 4) * 2 + hh
                                    dst = v_sb[:, st * VW + h * VB:
                                               st * VW + h * VB + HEAD]
                                    nc.sync.dma_start_transpose(
                                        dst, vtt[hh * 64:hh * 64 + 64,
                                                 u * 128:(u + 1) * 128])
                ps_qkv_cm.__exit__(None, None, None)

            # -------- Phase 2: banded attention + normalize + out-proj --------
            with tc.tile_pool(name="atp", bufs=1) as atp, \
                 tc.tile_pool(name="wo", bufs=1) as wop:
                oT_sb = [atp.tile([128, S], F32, tag=f"osb{p}", name=f"osb{p}")
                         for p in range(2)]
                onrm = [atp.tile([128, S], BF16, tag=f"onrm{p}", name=f"onrm{p}")
                        for p in range(2)]
                wo_sb = [wop.tile([128, SIZE], BF16, tag=f"wo{k}", name=f"wo{k}")
                         for k in range(2)]
                for k in range(2):
                    nc.gpsimd.dma_start(wo_sb[k], woutT_d[k * 128:(k + 1) * 128, :])

                ptiles = {}
                with tc.tile_pool(name="pt", bufs=9) as ptp, \
                     tc.tile_pool(name="stg", bufs=4) as stgp, \
                     tc.tile_pool(name="ibc", bufs=3) as ibcp, \
                     tc.tile_pool(name="ost", bufs=4) as ostp, \
                     tc.tile_pool(name="ps_sc", bufs=2, space="PSUM") as ps_sc, \
                     tc.tile_pool(name="ps_ot", bufs=1, space="PSUM") as ps_ot, \
                     tc.tile_pool(name="ps_op", bufs=2, space="PSUM") as ps_op:

                    def emit_group(qg):
                        # o-matmuls for q-tiles 4qg..4qg+3, k2-major so the
                        # stationary v block is reused across regions
                        t0, t3 = 4 * qg, 4 * qg + 3
                        klo_g, khi_g = max(t0 - 2, 0), min(t3 + 2, NT - 1)
                        ots = [ps_ot.tile([HEAD + 1, 512], F32, tag=f"ot{h}",
                                          name=f"ot{h}") for h in range(HL)]
                        # k2 = t0+1 attends the whole group (N=512): do it first
                        # with start=True since start clears the full psum bank.
                        k2s = [t0 + 1] + [k for k in range(klo_g, khi_g + 1)
                                          if k != t0 + 1]
                        for ki, k2 in enumerate(k2s):
                            ts = [t for t in range(t0, t3 + 1)
                                  if k2 - 2 <= t <= k2 + 2]
                            for h in range(HL):
                                pt, qlo2 = ptiles[(k2, h)]
                                vs = v_sb[:, k2 * VW + h * VB:
                                          k2 * VW + h * VB + HEAD + 1]
                                ja, jb = ts[0] - t0, ts[-1] - t0
                                off = (ts[0] - qlo2) * 128
                                nc.tensor.matmul(
                                    ots[h][:, ja * 128:(jb + 1) * 128],
                                    vs, pt[:, off:off + 128 * len(ts)],
                                    start=(ki == 0), stop=(ki == len(k2s) - 1),
                                    skip_group_check=True)
                        gsl = slice(qg * 512, (qg + 1) * 512)
                        stage_ws = []
                        for h in range(HL):
                            pair = h // 2
                            stage = stgp.tile([1, 512], F32, tag="stage")
                            nc.scalar.copy(stage, ots[h][HEAD:HEAD + 1, :])
                            sw = nc.sync.dma_start(sums_b[h, gsl], stage)
                            stage_ws.append(sw)
                            nc.scalar.copy(
                                oT_sb[pair][(h % 2) * 64:(h % 2) * 64 + 64, gsl],
                                ots[h][0:HEAD, :])
                        # reciprocal of the 4x512 sums as a (128,16) tile
                        sgth = stgp.tile([128, 16], F32, tag="sgth")
                        sgr = nc.sync.dma_start(
                            sgth, bass.AP(tensor=sums_b, offset=qg * 512,
                                          ap=[[S, 4], [16, 32], [1, 16]]))
                        for w in stage_ws:
                            _add_dep_helper(sgr.ins, w.ins, sync=True,
                                            reason="sums gather after stage writes")
                        sgti = stgp.tile([128, 16], F32, tag="sgti")
                        nc.vector.reciprocal(sgti, sgth)
                        invw = nc.sync.dma_start(
                            bass.AP(tensor=inv_b, offset=qg * 512,
                                    ap=[[S, 4], [16, 32], [1, 16]]), sgti)
                        for pair in range(2):
                            ibsb = ibcp.tile([128, 512], F32, tag="ibsb")
                            src_ap = bass.AP(tensor=inv_b,
                                             offset=pair * 2 * S + qg * 512,
                                             ap=[[S, 2], [0, 64], [1, 512]])
                            nc.sync.dma_start(ibsb, src_ap)
                            nc.vector.tensor_mul(onrm[pair][:, gsl],
                                                 oT_sb[pair][:, gsl], ibsb)
                        for ct in range(CT):
                            ops = ps_op.tile([128, 512], F32)
                            nc.tensor.matmul(ops, wo_sb[0][:, ct * 128:(ct + 1) * 128],
                                             onrm[0][:, gsl], start=True, stop=False)
                            nc.tensor.matmul(ops, wo_sb[1][:, ct * 128:(ct + 1) * 128],
                                             onrm[1][:, gsl], start=False, stop=True)
                            ot = ostp.tile([128, 512], F32)
                            if ct % 2 == 0:
                                nc.scalar.copy(ot, ops)
                            else:
                                nc.vector.tensor_copy(ot, ops)
                            nc.sync.dma_start(outT_d[ct * 128:(ct + 1) * 128, gsl], ot)

                    for kt in range(NT):
                        qlo, qhi = max(kt - 2, 0), min(kt + 2, NT - 1)
                        span = (qhi - qlo + 1) * 128
                        chunks = ([(0, 384), (384, span - 384)] if span > 512
                                  else [(0, span)])
                        for h in range(HL):
                            ktile = qk_sb[2 + h // 2][(h % 2) * 64:(h % 2) * 64 + 64,
                                                      kt * 128:(kt + 1) * 128]
                            pt = ptp.tile([128, 640], BF16, tag=f"pt{h}")
                            for (c0, w) in chunks:
                                sps = ps_sc.tile([128, 512], F32)
                                qtile = qk_sb[h // 2][
                                    (h % 2) * 64:(h % 2) * 64 + 64,
                                    qlo * 128 + c0: qlo * 128 + c0 + w]
                                nc.tensor.matmul(sps[:, :w], ktile, qtile,
                                                 start=True, stop=True)
                                nc.scalar.activation(pt[:, c0:c0 + w], sps[:, :w],
                                                     AF.Exp)
                            if kt >= 2:
                                nc.vector.tensor_mul(pt[:, 0:128], pt[:, 0:128], mlo)
                            if kt <= NT - 3:
                                nc.vector.tensor_mul(pt[:, span - 128:span],
                                                     pt[:, span - 128:span], mhi)
                            ptiles[(kt, h)] = (pt, qlo)
                        for qg in range(NT // 4):
                            if kt == min(4 * qg + 5, NT - 1):
                                emit_group(qg)
                    if dbg:
                        nc.sync.dma_start(dbg["d_inv"], inv_b[:, :])
                        nc.sync.dma_start(dbg["d_qk0"][:, 0:128], qk_sb[0][:, 0:128])
                        nc.sync.dma_start(dbg["d_vsb"][:, 0:128], v_sb[:, 0:128])
                        nc.sync.dma_start(dbg["d_osb0"][:, 0:128], oT_sb[0][:, 0:128])
                        nc.sync.dma_start(dbg["d_onrm0"][:, 0:128], onrm[0][:, 0:128])
            vtp_cm.__exit__(None, None, None)

    nc.compile()
    nc.m = get_hw_module(nc.m)
    return nc


def _rope_cos_sin(n, d):
    inv_freq = 1.0 / (ROPE_BASE ** (np.arange(0, d, 2, dtype=np.float32) / d))
    freqs = np.arange(n, dtype=np.float32)[:, None] * inv_freq[None, :]
    emb = np.concatenate([freqs, freqs], axis=-1)
    return np.cos(emb).astype(np.float32), np.sin(emb).astype(np.float32)


def _prep_inputs(x, w_qkv, w_out, norm_scale):
    cos, sin = _rope_cos_sin(HEADS, HEAD)
    Wf = np.ascontiguousarray(w_qkv.reshape(3, HEADS, HEAD, SIZE)).astype(np.float32)
    d2 = HEAD // 2

    def rot(Wh, h):
        out = np.empty_like(Wh)
        out[:d2] = cos[h, :d2, None] * Wh[:d2] - sin[h, :d2, None] * Wh[d2:]
        out[d2:] = cos[h, d2:, None] * Wh[d2:] + sin[h, d2:, None] * Wh[:d2]
        return out

    scale = np.float32(1.0 / np.sqrt(HEAD))
    Wq = np.stack([rot(Wf[0, h], h) for h in range(HEADS)]) * scale
    Wk = np.stack([rot(Wf[1, h], h) for h in range(HEADS)])
    Wv = Wf[2]
    ns = norm_scale.astype(np.float32)[None, None, :]
    Wq = Wq * ns
    Wk = Wk * ns
    Wv = Wv * ns

    i = np.arange(128)
    mlo = (i[:, None] <= i[None, :]).astype(ml_dtypes.bfloat16)
    mhi = (i[:, None] >= i[None, :]).astype(ml_dtypes.bfloat16)

    in_maps = []
    for core in range(8):
        b, g = divmod(core, 4)
        hs = slice(HL * g, HL * g + HL)
        wq_l = Wq[hs].reshape(HL * HEAD, SIZE)
        wk_l = Wk[hs].reshape(HL * HEAD, SIZE)
        wv_l = Wv[hs].reshape(HL * HEAD, SIZE)
        wqkvT = np.ascontiguousarray(np.concatenate([wq_l, wk_l, wv_l], 0).T)
        woutT = np.ascontiguousarray(
            w_out[:, HL * HEAD * g: HL * HEAD * (g + 1)].T.astype(np.float32))
        in_maps.append({
            "xT": np.ascontiguousarray(x[b].T).astype(ml_dtypes.bfloat16),
            "wqkvT": wqkvT.astype(ml_dtypes.bfloat16),
            "woutT": woutT.astype(ml_dtypes.bfloat16),
            "onescol": np.ones((128, 1), ml_dtypes.bfloat16),
            "onesrow": np.ones((1, 128), np.float32),
            "mlo": mlo,
            "mhi": mhi,
        })
    return in_maps


def _run(in_maps, trace=False, **kw):
    if "nc" not in _cached:
        _cached["nc"] = _build_module()
    return run_bass_kernel_spmd(_cached["nc"], in_maps, core_ids=list(range(8)),
                                trace=trace, **kw)


def kernel(x, mask, w_qkv, w_out, norm_scale):
    x = np.asarray(x)
    in_maps = _prep_inputs(x, np.asarray(w_qkv), np.asarray(w_out),
                           np.asarray(norm_scale))
    res = _run(in_maps)
    out = np.empty((B, S, SIZE), np.float32)
    for b in range(B):
        acc = res.results[b * 4]["outT"].copy()
        for g in range(1, 4):
            acc += res.results[b * 4 + g]["outT"]
        out[b] = acc.T + x[b]
    return out
